# revision 3
# baseline (speedup 1.0000x reference)
"""
CoordinationHistogram TRN2 kernel, v6: rebalanced engine split.

Same two-level one-hot matmul as kernel.py (v5), with:
- DVE op-count trimmed: q/r/z tensors written at final dtype directly
  (no separate copy ops); min(y,1) folded into the ACT relu chain via
  r2 = relu(1-y), vv = r2^2, w2 = 3-2*r2.
- H one-hot bins reallocated: DVE builds fewer, ACT more (ACT was
  under-loaded after losing the edge-math copies).
"""

import numpy as np

import concourse.tile as tile
from concourse import bacc, mybir
from concourse.bass_utils import run_bass_kernel_spmd

P = 128
NQ = 157
NATOMS = 20000
K = 16
E = 1_000_000
NCOL_FULL = 7813
TBLK = 208          # columns per block (bin-major window)
LSG = 14            # columns per Pool local_scatter group
NPG = 9             # local_scatter groups per block on Pool
DVE_BINS = 122      # H bins built on DVE
POOL_BINS = 3       # H bins built on GPSIMD; rest (32) on ACT

R1 = 4.4
INV2 = float(1.0 / (1.1 * 1.1))
PAD_ATOM = 20064
GRPD = 21           # DVE L-mask mega-tile group

F32 = mybir.dt.float32
BF16 = mybir.dt.bfloat16
I32 = mybir.dt.int32
I16 = mybir.dt.int16
OP = mybir.AluOpType
AF = mybir.ActivationFunctionType


def _emit_cols(nc, ncol, col, blk, iota_l, mpool, coords, npg=NPG):
    """Emit L masks + matmuls for one (deferred) block."""
    tb, rb, zf, zb, idx_blk, hview = blk
    npool = min(npg, tb // LSG)
    t = 0
    for gi in range(npool):
        lmega = mpool.tile([P, LSG * P], BF16, tag="lsmega")
        nc.gpsimd.local_scatter(lmega[:], zb[:, t:t + LSG],
                                idx_blk[:, t:t + LSG],
                                P, LSG * P, LSG)
        for i in range(LSG):
            nc.tensor.matmul(
                out=coords[:],
                lhsT=lmega[:, i * P:(i + 1) * P],
                rhs=hview[:, :, t + i],
                start=(col + i == 0), stop=(col + i == ncol - 1))
        col += LSG
        t += LSG
    while t < tb:
        g = min(GRPD, tb - t)
        lmega = mpool.tile([P, GRPD * P], BF16, tag="lmega")
        for i in range(g):
            nc.vector.tensor_scalar(
                lmega[:, i * P:(i + 1) * P], iota_l[:],
                rb[:, t + i:t + i + 1], zf[:, t + i:t + i + 1],
                op0=OP.is_equal, op1=OP.mult)
        for i in range(g):
            nc.tensor.matmul(
                out=coords[:],
                lhsT=lmega[:, i * P:(i + 1) * P],
                rhs=hview[:, :, t + i],
                start=(col + i == 0), stop=(col + i == ncol - 1))
        col += g
        t += g
    return col


def build_nc(ncol=NCOL_FULL):
    e_pad = ncol * P
    nc = bacc.Bacc("TRN2", target_bir_lowering=False, debug=False)
    nv = nc.dram_tensor("nv", [e_pad * 3], F32, kind="ExternalInput")
    fa = nc.dram_tensor("fa", [e_pad], I32, kind="ExternalInput")
    out = nc.dram_tensor("out", [1, K], F32, kind="ExternalOutput")

    blocks = []
    c = 0
    while c < ncol:
        tb = min(TBLK, ncol - c)
        blocks.append((c, tb))
        c += tb

    with tile.TileContext(nc) as tc:
        with (
            tc.tile_pool(name="const", bufs=1) as cpool,
            tc.tile_pool(name="io", bufs=2) as iopool,
            tc.tile_pool(name="work", bufs=2) as wpool,
            tc.tile_pool(name="hb", bufs=2) as hpool,
            tc.tile_pool(name="mask", bufs=4) as mpool,
            tc.tile_pool(name="psum", bufs=1, space="PSUM") as ppool,
        ):
            iota_l = cpool.tile([P, P], BF16)
            nc.gpsimd.iota(iota_l[:], pattern=[[1, P]], base=0,
                           channel_multiplier=0,
                           allow_small_or_imprecise_dtypes=True)
            off_row = cpool.tile([P, NPG * LSG], I16)
            for g in range(NPG):
                nc.gpsimd.iota(off_row[:, g * LSG:(g + 1) * LSG],
                               pattern=[[P, LSG]], base=0,
                               channel_multiplier=0,
                               allow_small_or_imprecise_dtypes=True)
            ones = cpool.tile([P, 1], F32)
            nc.vector.memset(ones[:], 1.0)
            bias_m4 = cpool.tile([P, 1], F32)
            nc.vector.memset(bias_m4[:], -4.0)
            # -j bias table for ACT-built bins
            bias_q = cpool.tile([P, NQ], F32)
            iq = cpool.tile([P, NQ], I16)
            nc.gpsimd.iota(iq[:], pattern=[[1, NQ]], base=0,
                           channel_multiplier=0)
            nc.vector.tensor_copy(bias_q[:], iq[:])
            nc.vector.tensor_scalar(bias_q[:], bias_q[:], -1.0, None,
                                    op0=OP.mult)

            coords = ppool.tile([P, NQ], F32, space="PSUM")

            col = 0
            prev = None
            for (c0, tb) in blocks:
                ofs_e = c0 * P
                nvb = iopool.tile([P, TBLK * 3], F32, tag="nvb")
                fab = iopool.tile([P, TBLK], I32, tag="fab")
                nc.sync.dma_start(
                    nvb[:, : tb * 3],
                    nv[ofs_e * 3: (ofs_e + P * tb) * 3].rearrange(
                        "(p m) -> p m", p=P),
                )
                nc.sync.dma_start(
                    fab[:, :tb],
                    fa[ofs_e: ofs_e + P * tb].rearrange("(p m) -> p m", p=P),
                )
                v3 = nvb[:, : tb * 3].rearrange("p (m c) -> p m c", c=3)
                x, y, w = v3[:, :, 0], v3[:, :, 1], v3[:, :, 2]

                # ---- d^2 on DVE (f32 TT ops) ----
                d2 = wpool.tile([P, TBLK], F32, tag="d2")
                t1 = wpool.tile([P, TBLK], F32, tag="t1")
                nc.vector.tensor_tensor(out=d2[:, :tb], in0=x, in1=x, op=OP.mult)
                nc.vector.tensor_tensor(out=t1[:, :tb], in0=y, in1=y, op=OP.mult)
                nc.vector.tensor_tensor(out=d2[:, :tb], in0=d2[:, :tb],
                                        in1=t1[:, :tb], op=OP.add)
                nc.vector.tensor_tensor(out=t1[:, :tb], in0=w, in1=w, op=OP.mult)
                nc.vector.tensor_tensor(out=d2[:, :tb], in0=d2[:, :tb],
                                        in1=t1[:, :tb], op=OP.add)
                # ---- switching function on ACT ----
                # sv = d/1.1 ; y0 = relu(sv-4) = relu((d-4.4)/1.1)
                # r2 = relu(1-y0) = 1-min(y0,1) ; vv = r2^2 = (y-1)^2 clamped
                sv = wpool.tile([P, TBLK], F32, tag="sv")
                nc.scalar.activation(sv[:, :tb], d2[:, :tb], AF.Sqrt, scale=INV2)
                y0 = wpool.tile([P, TBLK], F32, tag="y0")
                nc.scalar.activation(y0[:, :tb], sv[:, :tb], AF.Relu,
                                     bias=bias_m4[:])
                r2 = wpool.tile([P, TBLK], F32, tag="r2")
                nc.scalar.activation(r2[:, :tb], y0[:, :tb], AF.Relu,
                                     bias=ones[:], scale=-1.0)
                vv = wpool.tile([P, TBLK], F32, tag="vv")
                nc.scalar.activation(vv[:, :tb], r2[:, :tb], AF.Square)
                # w2 = 3 - 2*r2 = 1 + 2*min(y,1)  (DVE, fused mult+add)
                w2 = wpool.tile([P, TBLK], F32, tag="w2")
                nc.vector.tensor_scalar(w2[:, :tb], r2[:, :tb], -2.0, 3.0,
                                        op0=OP.mult, op1=OP.add)
                zf = wpool.tile([P, TBLK], F32, tag="zf")
                nc.vector.tensor_tensor(out=zf[:, :tb], in0=vv[:, :tb],
                                        in1=w2[:, :tb], op=OP.mult)
                # bf16 z only needed for the Pool local_scatter columns
                npg = min(NPG, tb // LSG)
                zb = wpool.tile([P, NPG * LSG], BF16, tag="zb")
                if npg:
                    nc.vector.tensor_copy(zb[:, :npg * LSG],
                                          zf[:, :npg * LSG])
                # ---- q/r extraction (bit ops cannot cast dtypes) ----
                qi = wpool.tile([P, TBLK], I32, tag="qi")
                nc.vector.tensor_scalar(qi[:, :tb], fab[:, :tb], 7, None,
                                        op0=OP.logical_shift_right)
                qb = wpool.tile([P, TBLK], BF16, tag="qb")
                nc.vector.tensor_copy(qb[:, :tb], qi[:, :tb])
                ri = wpool.tile([P, TBLK], I32, tag="ri")
                nc.vector.tensor_scalar(ri[:, :tb], fab[:, :tb], 127, None,
                                        op0=OP.bitwise_and)
                rb = wpool.tile([P, TBLK], F32, tag="rb")
                nc.vector.tensor_copy(rb[:, :tb], ri[:, :tb])
                r16 = wpool.tile([P, TBLK], I16, tag="r16")
                nc.vector.tensor_copy(r16[:, :tb], ri[:, :tb])
                idx_blk = wpool.tile([P, NPG * LSG], I16, tag="idxblk")
                if npg:
                    nc.vector.tensor_tensor(
                        out=idx_blk[:, :npg * LSG],
                        in0=r16[:, :npg * LSG],
                        in1=off_row[:, :npg * LSG], op=OP.add)

                # ---- software pipeline: previous block's L + matmuls ----
                if prev is not None:
                    col = _emit_cols(nc, ncol, col, prev, iota_l,
                                     mpool, coords)

                # ---- bin-major H: HB[p, j*TBLK + t] = (q[p,t] == j) ----
                hb = hpool.tile([P, NQ * TBLK], BF16, tag="hb")
                for j in range(NQ):
                    hslice = hb[:, j * TBLK: j * TBLK + tb]
                    if j < DVE_BINS:
                        nc.vector.tensor_scalar(hslice, qb[:, :tb], float(j),
                                                None, op0=OP.is_equal)
                    elif j < DVE_BINS + POOL_BINS:
                        nc.gpsimd.tensor_scalar(hslice, qb[:, :tb], float(j),
                                                None, op0=OP.is_equal)
                    else:
                        hsq = mpool.tile([P, TBLK], BF16, tag="hsq")
                        nc.scalar.activation(hsq[:, :tb], qb[:, :tb],
                                             AF.Square,
                                             bias=bias_q[:, j:j + 1])
                        nc.scalar.activation(hslice, hsq[:, :tb], AF.Relu,
                                             bias=ones[:], scale=-1.0)
                hview = hb[:].rearrange("p (j t) -> p j t", t=TBLK)
                prev = (tb, rb, zf, zb, idx_blk, hview)
            col = _emit_cols(nc, ncol, col, prev, iota_l,
                             mpool, coords, npg=3)

            # ---- KDE ----
            nc.vector.memset(coords[32:64, 156:157], 1.0e6)
            nc.vector.memset(coords[64:128, 156:157], 1.0e6)
            acc1 = cpool.tile([P, K], F32)
            for k in range(K):
                dk = wpool.tile([P, NQ], F32, tag="dk")
                nc.vector.tensor_scalar(dk[:], coords[:], float(-k), None,
                                        op0=OP.add)
                sq = wpool.tile([P, NQ], F32, tag="sq")
                nc.vector.tensor_tensor(out=sq[:], in0=dk[:], in1=dk[:],
                                        op=OP.mult)
                ek = wpool.tile([P, NQ], F32, tag="ek")
                nc.scalar.activation(ek[:], sq[:], AF.Exp,
                                     scale=-2.0, accum_out=acc1[:, k:k + 1])
            hist_ps = ppool.tile([1, K], F32, space="PSUM")
            nc.tensor.matmul(out=hist_ps[:], lhsT=ones[:], rhs=acc1[:],
                             start=True, stop=True)
            res = cpool.tile([1, K], F32)
            nc.vector.tensor_copy(res[:], hist_ps[:])
            nc.sync.dma_start(out[:], res[:])
    nc.compile()
    return nc


def _shard_inputs(neighbor_vectors, first_atom, ncol=NCOL_FULL):
    e_pad = ncol * P
    s = neighbor_vectors.shape[0]
    in_maps = []
    for i in range(s):
        nvs = np.asarray(neighbor_vectors[i], dtype=np.float32).reshape(-1, 3)
        fas = np.asarray(first_atom[i], dtype=np.int32).reshape(-1)
        n = min(e_pad, nvs.shape[0])
        nv_pad = np.empty((e_pad, 3), dtype=np.float32)
        nv_pad[:n] = nvs[:n]
        nv_pad[n:] = np.array([10.0, 0.0, 0.0], dtype=np.float32)
        fa_pad = np.full((e_pad,), PAD_ATOM, dtype=np.int32)
        fa_pad[:n] = fas[:n]
        in_maps.append({"nv": nv_pad.reshape(-1), "fa": fa_pad})
    return in_maps


def run(neighbor_vectors, first_atom, ncol=NCOL_FULL, trace=False):
    nc = build_nc(ncol)
    in_maps = _shard_inputs(neighbor_vectors, first_atom, ncol)
    br = run_bass_kernel_spmd(nc, in_maps, core_ids=list(range(len(in_maps))),
                              trace=trace)
    out = np.stack([br.results[i]["out"][0] for i in range(len(in_maps))])
    return out.astype(np.float32), br


def kernel(neighbor_vectors, first_atom):
    out, _ = run(neighbor_vectors, first_atom)
    return out


# revision 10
# speedup vs baseline: 1.0160x; 1.0160x over previous
"""
CoordinationHistogram TRN2 kernel, v6: rebalanced engine split.

Same two-level one-hot matmul as kernel.py (v5), with:
- DVE op-count trimmed: q/r/z tensors written at final dtype directly
  (no separate copy ops); min(y,1) folded into the ACT relu chain via
  r2 = relu(1-y), vv = r2^2, w2 = 3-2*r2.
- H one-hot bins reallocated: DVE builds fewer, ACT more (ACT was
  under-loaded after losing the edge-math copies).
"""

import numpy as np

import concourse.tile as tile
from concourse import bacc, mybir
from concourse.bass_utils import run_bass_kernel_spmd

P = 128
NQ = 157
NATOMS = 20000
K = 16
E = 1_000_000
NCOL_FULL = 7813
TBLK = 208          # columns per block (bin-major window)
LSG = 14            # columns per Pool local_scatter group
NPG = 9             # local_scatter groups per block on Pool
DVE_BINS = 125      # H bins built on DVE
POOL_BINS = 2       # H bins built on GPSIMD; rest (30) on ACT

R1 = 4.4
INV2 = float(1.0 / (1.1 * 1.1))
PAD_ATOM = 20064
GRPD = 21           # DVE L-mask mega-tile group

F32 = mybir.dt.float32
BF16 = mybir.dt.bfloat16
I32 = mybir.dt.int32
I16 = mybir.dt.int16
OP = mybir.AluOpType
AF = mybir.ActivationFunctionType


def _emit_cols(nc, ncol, col, blk, iota_l, mpool, coords, npg=NPG):
    """Emit L masks + matmuls for one (deferred) block."""
    tb, rb, zf, zb, idx_blk, hview = blk
    npool = min(npg, tb // LSG)
    t = 0
    for gi in range(npool):
        lmega = mpool.tile([P, LSG * P], BF16, tag="lsmega")
        nc.gpsimd.local_scatter(lmega[:], zb[:, t:t + LSG],
                                idx_blk[:, t:t + LSG],
                                P, LSG * P, LSG)
        for i in range(LSG):
            nc.tensor.matmul(
                out=coords[:],
                lhsT=lmega[:, i * P:(i + 1) * P],
                rhs=hview[:, :, t + i],
                start=(col + i == 0), stop=(col + i == ncol - 1))
        col += LSG
        t += LSG
    while t < tb:
        g = min(GRPD, tb - t)
        lmega = mpool.tile([P, GRPD * P], BF16, tag="lmega")
        for i in range(g):
            nc.vector.tensor_scalar(
                lmega[:, i * P:(i + 1) * P], iota_l[:],
                rb[:, t + i:t + i + 1], zf[:, t + i:t + i + 1],
                op0=OP.is_equal, op1=OP.mult)
        for i in range(g):
            nc.tensor.matmul(
                out=coords[:],
                lhsT=lmega[:, i * P:(i + 1) * P],
                rhs=hview[:, :, t + i],
                start=(col + i == 0), stop=(col + i == ncol - 1))
        col += g
        t += g
    return col


def build_nc(ncol=NCOL_FULL):
    e_pad = ncol * P
    nc = bacc.Bacc("TRN2", target_bir_lowering=False, debug=False)
    nv = nc.dram_tensor("nv", [e_pad * 3], F32, kind="ExternalInput")
    fa = nc.dram_tensor("fa", [e_pad], I32, kind="ExternalInput")
    out = nc.dram_tensor("out", [1, K], F32, kind="ExternalOutput")

    blocks = []
    c = 0
    while c < ncol:
        tb = min(TBLK, ncol - c)
        blocks.append((c, tb))
        c += tb

    with tile.TileContext(nc) as tc:
        with (
            tc.tile_pool(name="const", bufs=1) as cpool,
            tc.tile_pool(name="io", bufs=2) as iopool,
            tc.tile_pool(name="work", bufs=2) as wpool,
            tc.tile_pool(name="hb", bufs=2) as hpool,
            tc.tile_pool(name="mask", bufs=4) as mpool,
            tc.tile_pool(name="psum", bufs=1, space="PSUM") as ppool,
        ):
            iota_l = cpool.tile([P, P], BF16)
            nc.gpsimd.iota(iota_l[:], pattern=[[1, P]], base=0,
                           channel_multiplier=0,
                           allow_small_or_imprecise_dtypes=True)
            off_row = cpool.tile([P, NPG * LSG], I16)
            for g in range(NPG):
                nc.gpsimd.iota(off_row[:, g * LSG:(g + 1) * LSG],
                               pattern=[[P, LSG]], base=0,
                               channel_multiplier=0,
                               allow_small_or_imprecise_dtypes=True)
            ones = cpool.tile([P, 1], F32)
            nc.vector.memset(ones[:], 1.0)
            bias_m4 = cpool.tile([P, 1], F32)
            nc.vector.memset(bias_m4[:], -4.0)
            # -j bias table for ACT-built bins
            bias_q = cpool.tile([P, NQ], F32)
            iq = cpool.tile([P, NQ], I16)
            nc.gpsimd.iota(iq[:], pattern=[[1, NQ]], base=0,
                           channel_multiplier=0)
            nc.vector.tensor_copy(bias_q[:], iq[:])
            nc.vector.tensor_scalar(bias_q[:], bias_q[:], -1.0, None,
                                    op0=OP.mult)

            coords = ppool.tile([P, NQ], F32, space="PSUM")

            col = 0
            prev = None
            for (c0, tb) in blocks:
                ofs_e = c0 * P
                nvb = iopool.tile([P, TBLK * 3], F32, tag="nvb")
                fab = iopool.tile([P, TBLK], I32, tag="fab")
                nc.sync.dma_start(
                    nvb[:, : tb * 3],
                    nv[ofs_e * 3: (ofs_e + P * tb) * 3].rearrange(
                        "(p m) -> p m", p=P),
                )
                nc.sync.dma_start(
                    fab[:, :tb],
                    fa[ofs_e: ofs_e + P * tb].rearrange("(p m) -> p m", p=P),
                )
                # ---- d^2: one ACT Square over interleaved xyz, 2 DVE adds ----
                sq = wpool.tile([P, TBLK * 3], F32, tag="sq3")
                h3 = (tb * 3) // 2
                nc.scalar.activation(sq[:, :h3], nvb[:, :h3], AF.Square)
                nc.scalar.activation(sq[:, h3: tb * 3], nvb[:, h3: tb * 3],
                                     AF.Square)
                s3 = sq[:, : tb * 3].rearrange("p (m c) -> p m c", c=3)
                d2 = wpool.tile([P, TBLK], F32, tag="d2")
                nc.vector.tensor_tensor(out=d2[:, :tb], in0=s3[:, :, 0],
                                        in1=s3[:, :, 1], op=OP.add)
                nc.vector.tensor_tensor(out=d2[:, :tb], in0=d2[:, :tb],
                                        in1=s3[:, :, 2], op=OP.add)
                # ---- switching function on ACT ----
                # sv = d/1.1 ; y0 = relu(sv-4) = relu((d-4.4)/1.1)
                # r2 = relu(1-y0) = 1-min(y0,1) ; vv = r2^2 = (y-1)^2 clamped
                sv = wpool.tile([P, TBLK], F32, tag="sv")
                nc.scalar.activation(sv[:, :tb], d2[:, :tb], AF.Sqrt, scale=INV2)
                y0 = wpool.tile([P, TBLK], F32, tag="y0")
                nc.scalar.activation(y0[:, :tb], sv[:, :tb], AF.Relu,
                                     bias=bias_m4[:])
                r2 = wpool.tile([P, TBLK], F32, tag="r2")
                nc.scalar.activation(r2[:, :tb], y0[:, :tb], AF.Relu,
                                     bias=ones[:], scale=-1.0)
                vv = wpool.tile([P, TBLK], F32, tag="vv")
                nc.scalar.activation(vv[:, :tb], r2[:, :tb], AF.Square)
                # w2 = 3 - 2*r2 = 1 + 2*min(y,1)  (DVE, fused mult+add)
                w2 = wpool.tile([P, TBLK], F32, tag="w2")
                nc.vector.tensor_scalar(w2[:, :tb], r2[:, :tb], -2.0, 3.0,
                                        op0=OP.mult, op1=OP.add)
                zf = wpool.tile([P, TBLK], F32, tag="zf")
                nc.vector.tensor_tensor(out=zf[:, :tb], in0=vv[:, :tb],
                                        in1=w2[:, :tb], op=OP.mult)
                # bf16 z only needed for the Pool local_scatter columns
                npg = min(NPG, tb // LSG)
                zb = wpool.tile([P, NPG * LSG], BF16, tag="zb")
                if npg:
                    nc.vector.tensor_copy(zb[:, :npg * LSG],
                                          zf[:, :npg * LSG])
                # ---- q/r extraction (bit ops cannot cast dtypes) ----
                qi = wpool.tile([P, TBLK], I32, tag="qi")
                nc.vector.tensor_scalar(qi[:, :tb], fab[:, :tb], 7, None,
                                        op0=OP.logical_shift_right)
                qb = wpool.tile([P, TBLK], BF16, tag="qb")
                nc.vector.tensor_copy(qb[:, :tb], qi[:, :tb])
                ri = wpool.tile([P, TBLK], I32, tag="ri")
                nc.vector.tensor_scalar(ri[:, :tb], fab[:, :tb], 127, None,
                                        op0=OP.bitwise_and)
                rb = wpool.tile([P, TBLK], F32, tag="rb")
                nc.vector.tensor_copy(rb[:, :tb], ri[:, :tb])
                r16 = wpool.tile([P, TBLK], I16, tag="r16")
                nc.vector.tensor_copy(r16[:, :tb], ri[:, :tb])
                idx_blk = wpool.tile([P, NPG * LSG], I16, tag="idxblk")
                if npg:
                    nc.vector.tensor_tensor(
                        out=idx_blk[:, :npg * LSG],
                        in0=r16[:, :npg * LSG],
                        in1=off_row[:, :npg * LSG], op=OP.add)

                # ---- software pipeline: previous block's L + matmuls ----
                if prev is not None:
                    col = _emit_cols(nc, ncol, col, prev, iota_l,
                                     mpool, coords)

                # ---- bin-major H: HB[p, j*TBLK + t] = (q[p,t] == j) ----
                hb = hpool.tile([P, NQ * TBLK], BF16, tag="hb")
                for j in range(NQ):
                    hslice = hb[:, j * TBLK: j * TBLK + tb]
                    if j < DVE_BINS:
                        nc.vector.tensor_scalar(hslice, qb[:, :tb], float(j),
                                                None, op0=OP.is_equal)
                    elif j < DVE_BINS + POOL_BINS:
                        nc.gpsimd.tensor_scalar(hslice, qb[:, :tb], float(j),
                                                None, op0=OP.is_equal)
                    else:
                        hsq = mpool.tile([P, TBLK], BF16, tag="hsq")
                        nc.scalar.activation(hsq[:, :tb], qb[:, :tb],
                                             AF.Square,
                                             bias=bias_q[:, j:j + 1])
                        nc.scalar.activation(hslice, hsq[:, :tb], AF.Relu,
                                             bias=ones[:], scale=-1.0)
                hview = hb[:].rearrange("p (j t) -> p j t", t=TBLK)
                prev = (tb, rb, zf, zb, idx_blk, hview)
            col = _emit_cols(nc, ncol, col, prev, iota_l,
                             mpool, coords, npg=3)

            # ---- KDE ----
            nc.vector.memset(coords[32:64, 156:157], 1.0e6)
            nc.vector.memset(coords[64:128, 156:157], 1.0e6)
            acc1 = cpool.tile([P, K], F32)
            for k in range(K):
                dk = wpool.tile([P, NQ], F32, tag="dk")
                nc.vector.tensor_scalar(dk[:], coords[:], float(-k), None,
                                        op0=OP.add)
                sq = wpool.tile([P, NQ], F32, tag="sq")
                nc.vector.tensor_tensor(out=sq[:], in0=dk[:], in1=dk[:],
                                        op=OP.mult)
                ek = wpool.tile([P, NQ], F32, tag="ek")
                nc.scalar.activation(ek[:], sq[:], AF.Exp,
                                     scale=-2.0, accum_out=acc1[:, k:k + 1])
            hist_ps = ppool.tile([1, K], F32, space="PSUM")
            nc.tensor.matmul(out=hist_ps[:], lhsT=ones[:], rhs=acc1[:],
                             start=True, stop=True)
            res = cpool.tile([1, K], F32)
            nc.vector.tensor_copy(res[:], hist_ps[:])
            nc.sync.dma_start(out[:], res[:])
    nc.compile()
    return nc


def _shard_inputs(neighbor_vectors, first_atom, ncol=NCOL_FULL):
    e_pad = ncol * P
    s = neighbor_vectors.shape[0]
    in_maps = []
    for i in range(s):
        nvs = np.asarray(neighbor_vectors[i], dtype=np.float32).reshape(-1, 3)
        fas = np.asarray(first_atom[i], dtype=np.int32).reshape(-1)
        n = min(e_pad, nvs.shape[0])
        nv_pad = np.empty((e_pad, 3), dtype=np.float32)
        nv_pad[:n] = nvs[:n]
        nv_pad[n:] = np.array([10.0, 0.0, 0.0], dtype=np.float32)
        fa_pad = np.full((e_pad,), PAD_ATOM, dtype=np.int32)
        fa_pad[:n] = fas[:n]
        in_maps.append({"nv": nv_pad.reshape(-1), "fa": fa_pad})
    return in_maps


def run(neighbor_vectors, first_atom, ncol=NCOL_FULL, trace=False):
    nc = build_nc(ncol)
    in_maps = _shard_inputs(neighbor_vectors, first_atom, ncol)
    br = run_bass_kernel_spmd(nc, in_maps, core_ids=list(range(len(in_maps))),
                              trace=trace)
    out = np.stack([br.results[i]["out"][0] for i in range(len(in_maps))])
    return out.astype(np.float32), br


def kernel(neighbor_vectors, first_atom):
    out, _ = run(neighbor_vectors, first_atom)
    return out


# revision 18
# speedup vs baseline: 1.0206x; 1.0046x over previous
"""
CoordinationHistogram TRN2 kernel, v6: rebalanced engine split.

Same two-level one-hot matmul as kernel.py (v5), with:
- DVE op-count trimmed: q/r/z tensors written at final dtype directly
  (no separate copy ops); min(y,1) folded into the ACT relu chain via
  r2 = relu(1-y), vv = r2^2, w2 = 3-2*r2.
- H one-hot bins reallocated: DVE builds fewer, ACT more (ACT was
  under-loaded after losing the edge-math copies).
"""

import numpy as np

import concourse.tile as tile
from concourse import bacc, mybir
from concourse.bass_utils import run_bass_kernel_spmd

P = 128
NQ = 157
NATOMS = 20000
K = 16
E = 1_000_000
NCOL_FULL = 7813
TBLK = 208          # columns per block (bin-major window)
LSG = 14            # columns per Pool local_scatter group
NPG = 9             # local_scatter groups per block on Pool
DVE_BINS = 125      # H bins built on DVE
POOL_BINS = 2       # H bins built on GPSIMD; rest (30) on ACT

R1 = 4.4
INV2 = float(1.0 / (1.1 * 1.1))
PAD_ATOM = 20064
GRPD = 19           # DVE L-mask mega-tile group

F32 = mybir.dt.float32
BF16 = mybir.dt.bfloat16
I32 = mybir.dt.int32
I16 = mybir.dt.int16
OP = mybir.AluOpType
AF = mybir.ActivationFunctionType


def _emit_cols(nc, ncol, col, blk, iota_l, mpool, coords, npg=NPG):
    """Emit L masks + matmuls for one (deferred) block."""
    tb, rb, zf, zb, idx_blk, hview = blk
    npool = min(npg, tb // LSG)
    t = 0
    for gi in range(npool):
        lmega = mpool.tile([P, LSG * P], BF16, tag="lsmega")
        nc.gpsimd.local_scatter(lmega[:], zb[:, t:t + LSG],
                                idx_blk[:, t:t + LSG],
                                P, LSG * P, LSG)
        for i in range(LSG):
            nc.tensor.matmul(
                out=coords[:],
                lhsT=lmega[:, i * P:(i + 1) * P],
                rhs=hview[:, :, t + i],
                start=(col + i == 0), stop=(col + i == ncol - 1))
        col += LSG
        t += LSG
    while t < tb:
        g = min(GRPD, tb - t)
        lmega = mpool.tile([P, GRPD * P], BF16, tag="lmega")
        for i in range(g):
            nc.vector.tensor_scalar(
                lmega[:, i * P:(i + 1) * P], iota_l[:],
                rb[:, t + i:t + i + 1], zf[:, t + i:t + i + 1],
                op0=OP.is_equal, op1=OP.mult)
        for i in range(g):
            nc.tensor.matmul(
                out=coords[:],
                lhsT=lmega[:, i * P:(i + 1) * P],
                rhs=hview[:, :, t + i],
                start=(col + i == 0), stop=(col + i == ncol - 1))
        col += g
        t += g
    return col


def build_nc(ncol=NCOL_FULL):
    e_pad = ncol * P
    nc = bacc.Bacc("TRN2", target_bir_lowering=False, debug=False)
    nv = nc.dram_tensor("nv", [e_pad * 3], F32, kind="ExternalInput")
    fa = nc.dram_tensor("fa", [e_pad], I32, kind="ExternalInput")
    out = nc.dram_tensor("out", [1, K], F32, kind="ExternalOutput")

    blocks = []
    c = 0
    while c < ncol:
        tb = min(TBLK, ncol - c)
        blocks.append((c, tb))
        c += tb

    with tile.TileContext(nc) as tc:
        with (
            tc.tile_pool(name="const", bufs=1) as cpool,
            tc.tile_pool(name="io", bufs=2) as iopool,
            tc.tile_pool(name="work", bufs=2) as wpool,
            tc.tile_pool(name="hb", bufs=2) as hpool,
            tc.tile_pool(name="mask", bufs=5) as mpool,
            tc.tile_pool(name="psum", bufs=1, space="PSUM") as ppool,
        ):
            iota_l = cpool.tile([P, P], BF16)
            nc.gpsimd.iota(iota_l[:], pattern=[[1, P]], base=0,
                           channel_multiplier=0,
                           allow_small_or_imprecise_dtypes=True)
            off_row = cpool.tile([P, NPG * LSG], I16)
            for g in range(NPG):
                nc.gpsimd.iota(off_row[:, g * LSG:(g + 1) * LSG],
                               pattern=[[P, LSG]], base=0,
                               channel_multiplier=0,
                               allow_small_or_imprecise_dtypes=True)
            ones = cpool.tile([P, 1], F32)
            nc.vector.memset(ones[:], 1.0)
            bias_m4 = cpool.tile([P, 1], F32)
            nc.vector.memset(bias_m4[:], -4.0)
            # -j bias table for ACT-built bins
            bias_q = cpool.tile([P, NQ], F32)
            iq = cpool.tile([P, NQ], I16)
            nc.gpsimd.iota(iq[:], pattern=[[1, NQ]], base=0,
                           channel_multiplier=0)
            nc.vector.tensor_copy(bias_q[:], iq[:])
            nc.vector.tensor_scalar(bias_q[:], bias_q[:], -1.0, None,
                                    op0=OP.mult)

            coords = ppool.tile([P, NQ], F32, space="PSUM")

            col = 0
            prev = None
            for (c0, tb) in blocks:
                ofs_e = c0 * P
                nvb = iopool.tile([P, TBLK * 3], F32, tag="nvb")
                fab = iopool.tile([P, TBLK], I32, tag="fab")
                nc.sync.dma_start(
                    nvb[:, : tb * 3],
                    nv[ofs_e * 3: (ofs_e + P * tb) * 3].rearrange(
                        "(p m) -> p m", p=P),
                )
                nc.sync.dma_start(
                    fab[:, :tb],
                    fa[ofs_e: ofs_e + P * tb].rearrange("(p m) -> p m", p=P),
                )
                # ---- d^2: one ACT Square over interleaved xyz, 2 DVE adds ----
                sq = wpool.tile([P, TBLK * 3], F32, tag="sq3")
                h3 = (tb * 3) // 2
                nc.scalar.activation(sq[:, :h3], nvb[:, :h3], AF.Square)
                nc.scalar.activation(sq[:, h3: tb * 3], nvb[:, h3: tb * 3],
                                     AF.Square)
                s3 = sq[:, : tb * 3].rearrange("p (m c) -> p m c", c=3)
                d2 = wpool.tile([P, TBLK], F32, tag="d2")
                nc.vector.tensor_tensor(out=d2[:, :tb], in0=s3[:, :, 0],
                                        in1=s3[:, :, 1], op=OP.add)
                nc.vector.tensor_tensor(out=d2[:, :tb], in0=d2[:, :tb],
                                        in1=s3[:, :, 2], op=OP.add)
                # ---- switching function on ACT ----
                # sv = d/1.1 ; y0 = relu(sv-4) = relu((d-4.4)/1.1)
                # r2 = relu(1-y0) = 1-min(y0,1) ; vv = r2^2 = (y-1)^2 clamped
                sv = wpool.tile([P, TBLK], F32, tag="sv")
                nc.scalar.activation(sv[:, :tb], d2[:, :tb], AF.Sqrt, scale=INV2)
                y0 = wpool.tile([P, TBLK], F32, tag="y0")
                nc.scalar.activation(y0[:, :tb], sv[:, :tb], AF.Relu,
                                     bias=bias_m4[:])
                r2 = wpool.tile([P, TBLK], F32, tag="r2")
                nc.scalar.activation(r2[:, :tb], y0[:, :tb], AF.Relu,
                                     bias=ones[:], scale=-1.0)
                vv = wpool.tile([P, TBLK], F32, tag="vv")
                nc.scalar.activation(vv[:, :tb], r2[:, :tb], AF.Square)
                # w2 = 3 - 2*r2 = 1 + 2*min(y,1)  (DVE, fused mult+add)
                w2 = wpool.tile([P, TBLK], F32, tag="w2")
                nc.vector.tensor_scalar(w2[:, :tb], r2[:, :tb], -2.0, 3.0,
                                        op0=OP.mult, op1=OP.add)
                zf = wpool.tile([P, TBLK], F32, tag="zf")
                nc.vector.tensor_tensor(out=zf[:, :tb], in0=vv[:, :tb],
                                        in1=w2[:, :tb], op=OP.mult)
                # bf16 z only needed for the Pool local_scatter columns
                npg = min(NPG, tb // LSG)
                zb = wpool.tile([P, NPG * LSG], BF16, tag="zb")
                if npg:
                    nc.vector.tensor_copy(zb[:, :npg * LSG],
                                          zf[:, :npg * LSG])
                # ---- q/r extraction (bit ops cannot cast dtypes) ----
                qi = wpool.tile([P, TBLK], I32, tag="qi")
                nc.vector.tensor_scalar(qi[:, :tb], fab[:, :tb], 7, None,
                                        op0=OP.logical_shift_right)
                qb = wpool.tile([P, TBLK], BF16, tag="qb")
                nc.vector.tensor_copy(qb[:, :tb], qi[:, :tb])
                ri = wpool.tile([P, TBLK], I32, tag="ri")
                nc.vector.tensor_scalar(ri[:, :tb], fab[:, :tb], 127, None,
                                        op0=OP.bitwise_and)
                rb = wpool.tile([P, TBLK], F32, tag="rb")
                nc.vector.tensor_copy(rb[:, :tb], ri[:, :tb])
                r16 = wpool.tile([P, TBLK], I16, tag="r16")
                nc.vector.tensor_copy(r16[:, :tb], ri[:, :tb])
                idx_blk = wpool.tile([P, NPG * LSG], I16, tag="idxblk")
                if npg:
                    nc.vector.tensor_tensor(
                        out=idx_blk[:, :npg * LSG],
                        in0=r16[:, :npg * LSG],
                        in1=off_row[:, :npg * LSG], op=OP.add)

                # ---- software pipeline: previous block's L + matmuls ----
                if prev is not None:
                    col = _emit_cols(nc, ncol, col, prev, iota_l,
                                     mpool, coords)

                # ---- bin-major H: HB[p, j*TBLK + t] = (q[p,t] == j) ----
                hb = hpool.tile([P, NQ * TBLK], BF16, tag="hb")
                for j in range(NQ):
                    hslice = hb[:, j * TBLK: j * TBLK + tb]
                    if j < DVE_BINS:
                        nc.vector.tensor_scalar(hslice, qb[:, :tb], float(j),
                                                None, op0=OP.is_equal)
                    elif j < DVE_BINS + POOL_BINS:
                        nc.gpsimd.tensor_scalar(hslice, qb[:, :tb], float(j),
                                                None, op0=OP.is_equal)
                    else:
                        hsq = mpool.tile([P, TBLK], BF16, tag="hsq")
                        nc.scalar.activation(hsq[:, :tb], qb[:, :tb],
                                             AF.Square,
                                             bias=bias_q[:, j:j + 1])
                        nc.scalar.activation(hslice, hsq[:, :tb], AF.Relu,
                                             bias=ones[:], scale=-1.0)
                hview = hb[:].rearrange("p (j t) -> p j t", t=TBLK)
                prev = (tb, rb, zf, zb, idx_blk, hview)
            col = _emit_cols(nc, ncol, col, prev, iota_l,
                             mpool, coords, npg=3)

            # ---- KDE ----
            nc.vector.memset(coords[32:64, 156:157], 1.0e6)
            nc.vector.memset(coords[64:128, 156:157], 1.0e6)
            acc1 = cpool.tile([P, K], F32)
            for k in range(K):
                dk = wpool.tile([P, NQ], F32, tag="dk")
                nc.vector.tensor_scalar(dk[:], coords[:], float(-k), None,
                                        op0=OP.add)
                sq = wpool.tile([P, NQ], F32, tag="sq")
                nc.vector.tensor_tensor(out=sq[:], in0=dk[:], in1=dk[:],
                                        op=OP.mult)
                ek = wpool.tile([P, NQ], F32, tag="ek")
                nc.scalar.activation(ek[:], sq[:], AF.Exp,
                                     scale=-2.0, accum_out=acc1[:, k:k + 1])
            hist_ps = ppool.tile([1, K], F32, space="PSUM")
            nc.tensor.matmul(out=hist_ps[:], lhsT=ones[:], rhs=acc1[:],
                             start=True, stop=True)
            res = cpool.tile([1, K], F32)
            nc.vector.tensor_copy(res[:], hist_ps[:])
            nc.sync.dma_start(out[:], res[:])
    nc.compile()
    return nc


def _shard_inputs(neighbor_vectors, first_atom, ncol=NCOL_FULL):
    e_pad = ncol * P
    s = neighbor_vectors.shape[0]
    in_maps = []
    for i in range(s):
        nvs = np.asarray(neighbor_vectors[i], dtype=np.float32).reshape(-1, 3)
        fas = np.asarray(first_atom[i], dtype=np.int32).reshape(-1)
        n = min(e_pad, nvs.shape[0])
        nv_pad = np.empty((e_pad, 3), dtype=np.float32)
        nv_pad[:n] = nvs[:n]
        nv_pad[n:] = np.array([10.0, 0.0, 0.0], dtype=np.float32)
        fa_pad = np.full((e_pad,), PAD_ATOM, dtype=np.int32)
        fa_pad[:n] = fas[:n]
        in_maps.append({"nv": nv_pad.reshape(-1), "fa": fa_pad})
    return in_maps


def run(neighbor_vectors, first_atom, ncol=NCOL_FULL, trace=False):
    nc = build_nc(ncol)
    in_maps = _shard_inputs(neighbor_vectors, first_atom, ncol)
    br = run_bass_kernel_spmd(nc, in_maps, core_ids=list(range(len(in_maps))),
                              trace=trace)
    out = np.stack([br.results[i]["out"][0] for i in range(len(in_maps))])
    return out.astype(np.float32), br


def kernel(neighbor_vectors, first_atom):
    out, _ = run(neighbor_vectors, first_atom)
    return out


# revision 19
# speedup vs baseline: 1.0213x; 1.0007x over previous
"""
CoordinationHistogram TRN2 kernel, v6: rebalanced engine split.

Same two-level one-hot matmul as kernel.py (v5), with:
- DVE op-count trimmed: q/r/z tensors written at final dtype directly
  (no separate copy ops); min(y,1) folded into the ACT relu chain via
  r2 = relu(1-y), vv = r2^2, w2 = 3-2*r2.
- H one-hot bins reallocated: DVE builds fewer, ACT more (ACT was
  under-loaded after losing the edge-math copies).
"""

import numpy as np

import concourse.tile as tile
from concourse import bacc, mybir
from concourse.bass_utils import run_bass_kernel_spmd

P = 128
NQ = 157
NATOMS = 20000
K = 16
E = 1_000_000
NCOL_FULL = 7813
TBLK = 208          # columns per block (bin-major window)
LSG = 14            # columns per Pool local_scatter group
NPG = 9             # local_scatter groups per block on Pool
DVE_BINS = 125      # H bins built on DVE
POOL_BINS = 2       # H bins built on GPSIMD; rest (30) on ACT

R1 = 4.4
INV2 = float(1.0 / (1.1 * 1.1))
PAD_ATOM = 20064
GRPD = 14           # DVE L-mask mega-tile group

F32 = mybir.dt.float32
BF16 = mybir.dt.bfloat16
I32 = mybir.dt.int32
I16 = mybir.dt.int16
OP = mybir.AluOpType
AF = mybir.ActivationFunctionType


def _emit_cols(nc, ncol, col, blk, iota_l, mpool, coords, npg=NPG):
    """Emit L masks + matmuls for one (deferred) block."""
    tb, rb, zf, zb, idx_blk, hview = blk
    npool = min(npg, tb // LSG)
    t = 0
    for gi in range(npool):
        lmega = mpool.tile([P, LSG * P], BF16, tag="lsmega")
        nc.gpsimd.local_scatter(lmega[:], zb[:, t:t + LSG],
                                idx_blk[:, t:t + LSG],
                                P, LSG * P, LSG)
        for i in range(LSG):
            nc.tensor.matmul(
                out=coords[:],
                lhsT=lmega[:, i * P:(i + 1) * P],
                rhs=hview[:, :, t + i],
                start=(col + i == 0), stop=(col + i == ncol - 1))
        col += LSG
        t += LSG
    while t < tb:
        g = min(GRPD, tb - t)
        lmega = mpool.tile([P, GRPD * P], BF16, tag="lmega")
        for i in range(g):
            nc.vector.tensor_scalar(
                lmega[:, i * P:(i + 1) * P], iota_l[:],
                rb[:, t + i:t + i + 1], zf[:, t + i:t + i + 1],
                op0=OP.is_equal, op1=OP.mult)
        for i in range(g):
            nc.tensor.matmul(
                out=coords[:],
                lhsT=lmega[:, i * P:(i + 1) * P],
                rhs=hview[:, :, t + i],
                start=(col + i == 0), stop=(col + i == ncol - 1))
        col += g
        t += g
    return col


def build_nc(ncol=NCOL_FULL):
    e_pad = ncol * P
    nc = bacc.Bacc("TRN2", target_bir_lowering=False, debug=False)
    nv = nc.dram_tensor("nv", [e_pad * 3], F32, kind="ExternalInput")
    fa = nc.dram_tensor("fa", [e_pad], I32, kind="ExternalInput")
    out = nc.dram_tensor("out", [1, K], F32, kind="ExternalOutput")

    blocks = []
    c = 0
    while c < ncol:
        tb = min(TBLK, ncol - c)
        blocks.append((c, tb))
        c += tb

    with tile.TileContext(nc) as tc:
        with (
            tc.tile_pool(name="const", bufs=1) as cpool,
            tc.tile_pool(name="io", bufs=2) as iopool,
            tc.tile_pool(name="work", bufs=2) as wpool,
            tc.tile_pool(name="hb", bufs=2) as hpool,
            tc.tile_pool(name="mask", bufs=6) as mpool,
            tc.tile_pool(name="psum", bufs=1, space="PSUM") as ppool,
        ):
            iota_l = cpool.tile([P, P], BF16)
            nc.gpsimd.iota(iota_l[:], pattern=[[1, P]], base=0,
                           channel_multiplier=0,
                           allow_small_or_imprecise_dtypes=True)
            off_row = cpool.tile([P, NPG * LSG], I16)
            for g in range(NPG):
                nc.gpsimd.iota(off_row[:, g * LSG:(g + 1) * LSG],
                               pattern=[[P, LSG]], base=0,
                               channel_multiplier=0,
                               allow_small_or_imprecise_dtypes=True)
            ones = cpool.tile([P, 1], F32)
            nc.vector.memset(ones[:], 1.0)
            bias_m4 = cpool.tile([P, 1], F32)
            nc.vector.memset(bias_m4[:], -4.0)
            # -j bias table for ACT-built bins
            bias_q = cpool.tile([P, NQ], F32)
            iq = cpool.tile([P, NQ], I16)
            nc.gpsimd.iota(iq[:], pattern=[[1, NQ]], base=0,
                           channel_multiplier=0)
            nc.vector.tensor_copy(bias_q[:], iq[:])
            nc.vector.tensor_scalar(bias_q[:], bias_q[:], -1.0, None,
                                    op0=OP.mult)

            coords = ppool.tile([P, NQ], F32, space="PSUM")

            col = 0
            prev = None
            for (c0, tb) in blocks:
                ofs_e = c0 * P
                nvb = iopool.tile([P, TBLK * 3], F32, tag="nvb")
                fab = iopool.tile([P, TBLK], I32, tag="fab")
                nc.sync.dma_start(
                    nvb[:, : tb * 3],
                    nv[ofs_e * 3: (ofs_e + P * tb) * 3].rearrange(
                        "(p m) -> p m", p=P),
                )
                nc.sync.dma_start(
                    fab[:, :tb],
                    fa[ofs_e: ofs_e + P * tb].rearrange("(p m) -> p m", p=P),
                )
                # ---- d^2: one ACT Square over interleaved xyz, 2 DVE adds ----
                sq = wpool.tile([P, TBLK * 3], F32, tag="sq3")
                h3 = (tb * 3) // 2
                nc.scalar.activation(sq[:, :h3], nvb[:, :h3], AF.Square)
                nc.scalar.activation(sq[:, h3: tb * 3], nvb[:, h3: tb * 3],
                                     AF.Square)
                s3 = sq[:, : tb * 3].rearrange("p (m c) -> p m c", c=3)
                d2 = wpool.tile([P, TBLK], F32, tag="d2")
                nc.vector.tensor_tensor(out=d2[:, :tb], in0=s3[:, :, 0],
                                        in1=s3[:, :, 1], op=OP.add)
                nc.vector.tensor_tensor(out=d2[:, :tb], in0=d2[:, :tb],
                                        in1=s3[:, :, 2], op=OP.add)
                # ---- switching function on ACT ----
                # sv = d/1.1 ; y0 = relu(sv-4) = relu((d-4.4)/1.1)
                # r2 = relu(1-y0) = 1-min(y0,1) ; vv = r2^2 = (y-1)^2 clamped
                sv = wpool.tile([P, TBLK], F32, tag="sv")
                nc.scalar.activation(sv[:, :tb], d2[:, :tb], AF.Sqrt, scale=INV2)
                y0 = wpool.tile([P, TBLK], F32, tag="y0")
                nc.scalar.activation(y0[:, :tb], sv[:, :tb], AF.Relu,
                                     bias=bias_m4[:])
                r2 = wpool.tile([P, TBLK], F32, tag="r2")
                nc.scalar.activation(r2[:, :tb], y0[:, :tb], AF.Relu,
                                     bias=ones[:], scale=-1.0)
                vv = wpool.tile([P, TBLK], F32, tag="vv")
                nc.scalar.activation(vv[:, :tb], r2[:, :tb], AF.Square)
                # w2 = 3 - 2*r2 = 1 + 2*min(y,1)  (DVE, fused mult+add)
                w2 = wpool.tile([P, TBLK], F32, tag="w2")
                nc.vector.tensor_scalar(w2[:, :tb], r2[:, :tb], -2.0, 3.0,
                                        op0=OP.mult, op1=OP.add)
                zf = wpool.tile([P, TBLK], F32, tag="zf")
                nc.vector.tensor_tensor(out=zf[:, :tb], in0=vv[:, :tb],
                                        in1=w2[:, :tb], op=OP.mult)
                # bf16 z only needed for the Pool local_scatter columns
                npg = min(NPG, tb // LSG)
                zb = wpool.tile([P, NPG * LSG], BF16, tag="zb")
                if npg:
                    nc.vector.tensor_copy(zb[:, :npg * LSG],
                                          zf[:, :npg * LSG])
                # ---- q/r extraction (bit ops cannot cast dtypes) ----
                qi = wpool.tile([P, TBLK], I32, tag="qi")
                nc.vector.tensor_scalar(qi[:, :tb], fab[:, :tb], 7, None,
                                        op0=OP.logical_shift_right)
                qb = wpool.tile([P, TBLK], BF16, tag="qb")
                nc.vector.tensor_copy(qb[:, :tb], qi[:, :tb])
                ri = wpool.tile([P, TBLK], I32, tag="ri")
                nc.vector.tensor_scalar(ri[:, :tb], fab[:, :tb], 127, None,
                                        op0=OP.bitwise_and)
                rb = wpool.tile([P, TBLK], F32, tag="rb")
                nc.vector.tensor_copy(rb[:, :tb], ri[:, :tb])
                r16 = wpool.tile([P, TBLK], I16, tag="r16")
                nc.vector.tensor_copy(r16[:, :tb], ri[:, :tb])
                idx_blk = wpool.tile([P, NPG * LSG], I16, tag="idxblk")
                if npg:
                    nc.vector.tensor_tensor(
                        out=idx_blk[:, :npg * LSG],
                        in0=r16[:, :npg * LSG],
                        in1=off_row[:, :npg * LSG], op=OP.add)

                # ---- software pipeline: previous block's L + matmuls ----
                if prev is not None:
                    col = _emit_cols(nc, ncol, col, prev, iota_l,
                                     mpool, coords)

                # ---- bin-major H: HB[p, j*TBLK + t] = (q[p,t] == j) ----
                hb = hpool.tile([P, NQ * TBLK], BF16, tag="hb")
                for j in range(NQ):
                    hslice = hb[:, j * TBLK: j * TBLK + tb]
                    if j < DVE_BINS:
                        nc.vector.tensor_scalar(hslice, qb[:, :tb], float(j),
                                                None, op0=OP.is_equal)
                    elif j < DVE_BINS + POOL_BINS:
                        nc.gpsimd.tensor_scalar(hslice, qb[:, :tb], float(j),
                                                None, op0=OP.is_equal)
                    else:
                        hsq = mpool.tile([P, TBLK], BF16, tag="hsq")
                        nc.scalar.activation(hsq[:, :tb], qb[:, :tb],
                                             AF.Square,
                                             bias=bias_q[:, j:j + 1])
                        nc.scalar.activation(hslice, hsq[:, :tb], AF.Relu,
                                             bias=ones[:], scale=-1.0)
                hview = hb[:].rearrange("p (j t) -> p j t", t=TBLK)
                prev = (tb, rb, zf, zb, idx_blk, hview)
            col = _emit_cols(nc, ncol, col, prev, iota_l,
                             mpool, coords, npg=3)

            # ---- KDE ----
            nc.vector.memset(coords[32:64, 156:157], 1.0e6)
            nc.vector.memset(coords[64:128, 156:157], 1.0e6)
            acc1 = cpool.tile([P, K], F32)
            for k in range(K):
                dk = wpool.tile([P, NQ], F32, tag="dk")
                nc.vector.tensor_scalar(dk[:], coords[:], float(-k), None,
                                        op0=OP.add)
                sq = wpool.tile([P, NQ], F32, tag="sq")
                nc.vector.tensor_tensor(out=sq[:], in0=dk[:], in1=dk[:],
                                        op=OP.mult)
                ek = wpool.tile([P, NQ], F32, tag="ek")
                nc.scalar.activation(ek[:], sq[:], AF.Exp,
                                     scale=-2.0, accum_out=acc1[:, k:k + 1])
            hist_ps = ppool.tile([1, K], F32, space="PSUM")
            nc.tensor.matmul(out=hist_ps[:], lhsT=ones[:], rhs=acc1[:],
                             start=True, stop=True)
            res = cpool.tile([1, K], F32)
            nc.vector.tensor_copy(res[:], hist_ps[:])
            nc.sync.dma_start(out[:], res[:])
    nc.compile()
    return nc


def _shard_inputs(neighbor_vectors, first_atom, ncol=NCOL_FULL):
    e_pad = ncol * P
    s = neighbor_vectors.shape[0]
    in_maps = []
    for i in range(s):
        nvs = np.asarray(neighbor_vectors[i], dtype=np.float32).reshape(-1, 3)
        fas = np.asarray(first_atom[i], dtype=np.int32).reshape(-1)
        n = min(e_pad, nvs.shape[0])
        nv_pad = np.empty((e_pad, 3), dtype=np.float32)
        nv_pad[:n] = nvs[:n]
        nv_pad[n:] = np.array([10.0, 0.0, 0.0], dtype=np.float32)
        fa_pad = np.full((e_pad,), PAD_ATOM, dtype=np.int32)
        fa_pad[:n] = fas[:n]
        in_maps.append({"nv": nv_pad.reshape(-1), "fa": fa_pad})
    return in_maps


def run(neighbor_vectors, first_atom, ncol=NCOL_FULL, trace=False):
    nc = build_nc(ncol)
    in_maps = _shard_inputs(neighbor_vectors, first_atom, ncol)
    br = run_bass_kernel_spmd(nc, in_maps, core_ids=list(range(len(in_maps))),
                              trace=trace)
    out = np.stack([br.results[i]["out"][0] for i in range(len(in_maps))])
    return out.astype(np.float32), br


def kernel(neighbor_vectors, first_atom):
    out, _ = run(neighbor_vectors, first_atom)
    return out
